# revision 22
# baseline (speedup 1.0000x reference)
"""Multi-head causal attention (B=4, S=2048, D=1024, H=16) on 8 TRN2 NeuronCores.

Sharding: data-parallel over batch (4) x tensor-parallel over heads (2 groups
of 8 heads) = 8 cores. Host sums the two head-group partials per batch (the
W_o row-shard all-reduce from the sharding hint, done during unshard).

Per core, one fused loop over the 4 sequence blocks sb:
  QKV(sb):  Q^T/K^T = (x @ Wq/Wk)^T chunks, V = x @ Wv   (fp32r matmuls,
            outputs cast to bf16 in the PSUM->SBUF copy)
  attn(iB=sb), flash-style with scores transposed [j, i]:
    per head-pair hp (heads at kt/qt partition rows 0:64 and 64:128, packed
    concurrently in the PE array via row groups):
      rect:  full 512-query scores chunks for key chunks jj < 4*iB
      diag:  the 4 causal 128x128 triangle blocks of both heads packed in one
             [128,1024] tile -> one exp + one affine_select for all 8 blocks
      rem:   the unmasked staircase remainder of the 4 diagonal key chunks
      AV:    accs[65, 512] += [V_h | ones]^T @ E  (row 64 = softmax denom)
      normalize: 1/denom broadcast (gpsimd) * attnout, odd head staged to
             partitions 64:127 by SBUF-to-SBUF DMA
  proj(iB): out[i,:] += anorm_hp^T @ W_o-chunk, summed over hp (bf16)

The interleaving keeps the PE busy during the ACT-bound exp stretches (QKV of
the next block fills PE idle slots) so the HAM clock gate stays at 8/8.
"""

import sys

if "/opt/trn_rl_repo" not in sys.path:
    sys.path.insert(0, "/opt/trn_rl_repo")

import numpy as np

import concourse.bacc as bacc
import concourse.mybir as mybir
import concourse.tile as tile
from concourse.bass import ts
from concourse.bass_utils import run_bass_kernel_spmd

F32 = mybir.dt.float32
F32R = mybir.dt.float32r
BF16 = mybir.dt.bfloat16
AF = mybir.ActivationFunctionType

B, S, D, H = 4, 2048, 1024, 16
HD = D // H           # 64
NCORES = 8
HG = 8                # heads per core
DC = HG * HD          # 512 feature cols per core
SB = 512              # s-block
NSB = S // SB         # 4
KC = D // 128         # 8 k-chunks
SCALE = 1.0 / np.sqrt(HD)

_cached_nc = None


def _build():
    nc = bacc.Bacc("TRN2", target_bir_lowering=False, debug=False)

    xt_d = nc.dram_tensor("xt", [D, S], F32R, kind="ExternalInput")      # x[b].T
    wq_d = nc.dram_tensor("wq", [D, DC], F32R, kind="ExternalInput")
    wk_d = nc.dram_tensor("wk", [D, DC], F32R, kind="ExternalInput")
    wv_d = nc.dram_tensor("wv", [D, DC], F32R, kind="ExternalInput")
    wot_d = nc.dram_tensor("wot", [DC, D], BF16, kind="ExternalInput")   # W_o[:, cols].T
    out_d = nc.dram_tensor("out", [S, D], F32, kind="ExternalOutput")

    from contextlib import ExitStack

    with tile.TileContext(nc) as tc:
        with ExitStack() as ctx:
            w_pool = ctx.enter_context(tc.tile_pool(name="wp", bufs=1))
            wot_pool = ctx.enter_context(tc.tile_pool(name="wotp", bufs=1))
            xt_pool = ctx.enter_context(tc.tile_pool(name="xtp", bufs=2))
            qt_pool = ctx.enter_context(tc.tile_pool(name="qtp", bufs=8))
            kt_pool = ctx.enter_context(tc.tile_pool(name="ktp", bufs=16))
            v_pool = ctx.enter_context(tc.tile_pool(name="vtp", bufs=16))
            e_pool = ctx.enter_context(tc.tile_pool(name="ep", bufs=8))
            t65_pool = ctx.enter_context(tc.tile_pool(name="t65p", bufs=4))
            an_pool = ctx.enter_context(tc.tile_pool(name="anp", bufs=8))
            alo_pool = ctx.enter_context(tc.tile_pool(name="alop", bufs=3))
            rec_pool = ctx.enter_context(tc.tile_pool(name="recp", bufs=6))
            bc_pool = ctx.enter_context(tc.tile_pool(name="bcp", bufs=4))
            o_pool = ctx.enter_context(tc.tile_pool(name="op", bufs=2))
            ps_big = ctx.enter_context(
                tc.tile_pool(name="ps_big", bufs=2, space="PSUM"))
            ps_acc = ctx.enter_context(
                tc.tile_pool(name="ps_acc", bufs=4, space="PSUM"))
            # ---- weights (persistent) ----
            wq = w_pool.tile([128, KC, DC], F32R, tag="wq")
            wk = w_pool.tile([128, KC, DC], F32R, tag="wk")
            wv = w_pool.tile([128, KC, DC], F32R, tag="wv")
            # single strided DMAs (one per tensor) -- per-chunk DMAs cost
            # ~650ns of Sync-engine issue time each
            xt_view = xt_d[:, :].rearrange("(kc p) s -> p kc s", p=128)
            xt_tiles = [None] * NSB
            xt_tiles[0] = xt_pool.tile([128, KC, SB], F32R, tag="xt", name="xt0")
            wq_view = wq_d[:, :].rearrange("(kc p) c -> p kc c", p=128)
            # split the startup-critical loads so the first Q matmuls can
            # begin after the first half lands
            nc.sync.dma_start(out=xt_tiles[0][:, 0:4, :], in_=xt_view[:, 0:4, 0:SB])
            nc.sync.dma_start(out=wq[:, 0:4, :], in_=wq_view[:, 0:4, :])
            nc.sync.dma_start(out=xt_tiles[0][:, 4:8, :], in_=xt_view[:, 4:8, 0:SB])
            nc.sync.dma_start(out=wq[:, 4:8, :], in_=wq_view[:, 4:8, :])
            nc.sync.dma_start(out=wk,
                              in_=wk_d[:, :].rearrange("(kc p) c -> p kc c", p=128))
            nc.sync.dma_start(out=wv,
                              in_=wv_d[:, :].rearrange("(kc p) c -> p kc c", p=128))
            wot_all = wot_pool.tile([128, 4, D], BF16, tag="wot")
            nc.sync.dma_start(out=wot_all,
                              in_=wot_d[:, :].rearrange("(t p) c -> p t c", p=128))
            wot = [wot_all[:, t, :] for t in range(4)]

            ones8 = v_pool.tile([128, HG], BF16, tag="ones8", bufs=1)
            nc.vector.memset(ones8, 1.0)

            # causal triangle mask, replicated 8x: mask[p, 128*s + c] =
            # (c >= p) ? 1 : 0 -- multiplied into the diag exp tiles
            msk128 = v_pool.tile([128, 128], F32, tag="msk128", bufs=1)
            nc.vector.memset(msk128, 1.0)
            nc.gpsimd.affine_select(
                out=msk128, in_=msk128, pattern=[[1, 128]],
                compare_op=mybir.AluOpType.is_ge,
                fill=0.0, base=0, channel_multiplier=-1,
            )
            mask8 = v_pool.tile([128, 1024], BF16, tag="mask8", bufs=1)
            for s8 in range(8):
                nc.vector.tensor_copy(mask8[:, ts(s8, 128)], msk128)

            # persistent per-(m, sb) K^T and per-chunk V tiles
            kt = {}   # (m, sb) -> [128, 512] bf16
            vt = {}   # jj -> [128, HG, HD+1] bf16

            for sb in range(NSB):
                iB = sb
                # ---------------- QKV for this s-block ----------------
                xt_t = xt_tiles[sb]

                qt_sb = {}
                for w_t, dst, wname in ((wq, qt_sb, "q"), (wk, kt, "k")):
                    for jp in range(2):
                        ps = ps_big.tile([128, 1024], F32, tag="big",
                                         name=f"qk_{wname}_{sb}_{jp}")
                        for half in range(2):
                            m = 2 * jp + half
                            for kc in range(KC):
                                nc.tensor.matmul(
                                    ps[:, ts(half, SB)],
                                    w_t[:, kc, ts(m, 128)],
                                    xt_t[:, kc, :],
                                    start=(kc == 0), stop=(kc == KC - 1),
                                )
                        for half in range(2):
                            m = 2 * jp + half
                            if wname == "q":
                                d_t = qt_pool.tile([128, SB], BF16, tag="qt",
                                                   name=f"qt_{m}_{sb}")
                                qt_sb[m] = d_t
                            else:
                                d_t = kt_pool.tile([128, SB], BF16, tag="kt",
                                                   name=f"kt_{m}_{sb}")
                                kt[(m, sb)] = d_t
                            nc.scalar.copy(d_t, ps[:, ts(half, SB)])
                for sp in range(2):
                    ps = ps_big.tile([128, 1024], F32, tag="big",
                                     name=f"v_{sb}_{sp}")
                    for half in range(2):
                        sc = 2 * sp + half
                        for kc in range(KC):
                            nc.tensor.matmul(
                                ps[:, ts(half, SB)],
                                xt_t[:, kc, ts(sc, 128)],
                                wv[:, kc, :],
                                start=(kc == 0), stop=(kc == KC - 1),
                            )
                    for half in range(2):
                        sc = 2 * sp + half
                        v_t = v_pool.tile([128, HG, HD + 1], BF16, tag="vt",
                                          name=f"vt{4 * sb + sc}")
                        vt[4 * sb + sc] = v_t
                        nc.scalar.copy(
                            v_t[:, :, 0:HD],
                            ps[:, ts(half, SB)].rearrange("p (h d) -> p h d", h=HG),
                        )
                        nc.vector.tensor_copy(v_t[:, :, HD:HD + 1], ones8)

                # prefetch next block's x^T while attention below runs
                if sb + 1 < NSB:
                    xt_tiles[sb + 1] = xt_pool.tile([128, KC, SB], F32R,
                                                    tag="xt", name=f"xt{sb + 1}")
                    nc.sync.dma_start(out=xt_tiles[sb + 1],
                                      in_=xt_view[:, :, ts(sb + 1, SB)])

                # ---------------- attention for query block iB ----------------
                # two head-pair chains run interleaved, stage by stage, so
                # each chain's PE work covers the other's exp/normalize
                # latency at pipeline boundaries (needs 4 acc banks)
                anorms = [None] * 4
                rem_layout = ((0, 0, 384), (384, 2, 128), (512, 1, 256))

                def make_stages(hp):
                    h0, h1 = 2 * hp, 2 * hp + 1
                    qt_t = qt_sb[hp]
                    acc = {
                        h0: ps_acc.tile([128, SB], F32, tag="acc",
                                        name=f"acc0_{iB}_{hp}"),
                        h1: ps_acc.tile([128, SB], F32, tag="acc",
                                        name=f"acc1_{iB}_{hp}"),
                    }

                    def rect(jj=0):
                        kt_t = kt[(hp, jj // 4)]
                        jl = jj % 4
                        ps = ps_big.tile([128, 1024], F32, tag="big",
                                         name=f"sc_{iB}_{hp}_{jj}")
                        nc.tensor.matmul(ps[:, 0:SB],
                                         kt_t[0:64, ts(jl, 128)],
                                         qt_t[0:64, :], start=True, stop=True)
                        nc.tensor.matmul(ps[:, SB:1024],
                                         kt_t[64:128, ts(jl, 128)],
                                         qt_t[64:128, :], start=True, stop=True)
                        e_t = e_pool.tile([128, 1024], BF16, tag="e",
                                          name=f"e_{iB}_{hp}_{jj}")
                        nc.scalar.activation(e_t, ps, AF.Exp, scale=float(SCALE))
                        first = jj == 0
                        nc.tensor.matmul(acc[h0][0:HD + 1, :],
                                         vt[jj][:, h0, :], e_t[:, 0:SB],
                                         start=first, stop=False,
                                         skip_group_check=True)
                        nc.tensor.matmul(acc[h1][0:HD + 1, :],
                                         vt[jj][:, h1, :], e_t[:, SB:1024],
                                         start=first, stop=False,
                                         skip_group_check=True)

                    def diag():
                        # h0 triangles -> bank 0, h1 -> bank 1: MMs on
                        # different PE row groups must not write the same
                        # PSUM bank (HW hang). start=True only on each
                        # bank's first MM; the rest land on pending-zero
                        # bytes and overwrite.
                        dg = ps_big.tile([128, 1024], F32, tag="big",
                                         name=f"dg_{iB}_{hp}")
                        kt_t = kt[(hp, iB)]
                        for t0 in range(4):
                            nc.tensor.matmul(dg[:, 128 * t0:128 * t0 + 128],
                                             kt_t[0:64, ts(t0, 128)],
                                             qt_t[0:64, ts(t0, 128)],
                                             start=(t0 == 0), stop=True,
                                             skip_group_check=True)
                            nc.tensor.matmul(
                                dg[:, 512 + 128 * t0:512 + 128 * t0 + 128],
                                kt_t[64:128, ts(t0, 128)],
                                qt_t[64:128, ts(t0, 128)],
                                start=(t0 == 0), stop=True,
                                skip_group_check=True)
                        e_d = e_pool.tile([128, 1024], BF16, tag="e",
                                          name=f"ed_{iB}_{hp}")
                        nc.scalar.activation(e_d, dg, AF.Exp, scale=float(SCALE))
                        nc.vector.tensor_mul(e_d, e_d, mask8)
                        for t0 in range(4):
                            jj = 4 * iB + t0
                            for h, rb in ((h0, 0), (h1, 512)):
                                nc.tensor.matmul(
                                    acc[h][0:HD + 1, ts(t0, 128)],
                                    vt[jj][:, h, :],
                                    e_d[:, rb + 128 * t0:rb + 128 * t0 + 128],
                                    start=(iB == 0 and t0 == 0), stop=False,
                                    skip_group_check=True)

                    def rem():
                        kt_t = kt[(hp, iB)]
                        rems = {}
                        for h, rb in ((h0, 0), (h1, 64)):
                            ps = ps_big.tile([128, 768], F32, tag="big",
                                             name=f"rm_{iB}_{hp}_{h}")
                            rems[h] = ps
                            # first MM per 2KB bank: coff=0 -> bank 0,
                            # coff=512 -> bank 1; coff=384 overwrites
                            # pending bytes.
                            for (coff, t0, N) in rem_layout:
                                qoff = 128 * (t0 + 1)
                                nc.tensor.matmul(ps[:, coff:coff + N],
                                                 kt_t[rb:rb + 64, ts(t0, 128)],
                                                 qt_t[rb:rb + 64, qoff:qoff + N],
                                                 start=(coff != 384), stop=True,
                                                 skip_group_check=True)
                        for h in (h0, h1):
                            e_r = e_pool.tile([128, 768], BF16, tag="e",
                                              name=f"er_{iB}_{hp}_{h}")
                            nc.scalar.activation(e_r, rems[h], AF.Exp,
                                                 scale=float(SCALE))
                            for ri, (coff, t0, N) in enumerate(rem_layout):
                                qoff = 128 * (t0 + 1)
                                nc.tensor.matmul(
                                    acc[h][0:HD + 1, qoff:qoff + N],
                                    vt[4 * iB + t0][:, h, :],
                                    e_r[:, coff:coff + N],
                                    start=False,
                                    stop=(ri == len(rem_layout) - 1),
                                    skip_group_check=True)

                    def norm():
                        t65 = {}
                        for h in (h0, h1):
                            t_t = t65_pool.tile([HD + 1, SB], F32, tag="t65",
                                                name=f"t65_{iB}_{h}")
                            t65[h] = t_t
                            nc.vector.tensor_copy(t_t, acc[h][0:HD + 1, :])
                        anp = an_pool.tile([128, SB], BF16, tag="an",
                                           name=f"an_{iB}_{hp}")
                        anorms[hp] = anp
                        for h in (h0, h1):
                            r_t = rec_pool.tile([1, SB], F32, tag="r",
                                                name=f"r_{iB}_{h}")
                            nc.sync.dma_start(out=r_t, in_=t65[h][HD:HD + 1, :])
                            rec_t = rec_pool.tile([1, SB], F32, tag="rec",
                                                  name=f"rec_{iB}_{h}")
                            nc.vector.reciprocal_approx_fast(out=rec_t, in_=r_t)
                            bc_t = bc_pool.tile([HD, SB], F32, tag="bc",
                                                name=f"bc_{iB}_{h}")
                            nc.gpsimd.partition_broadcast(bc_t, rec_t)
                            if h == h0:
                                nc.vector.tensor_mul(anp[0:HD, :],
                                                     t65[h][0:HD, :], bc_t)
                            else:
                                alo = alo_pool.tile([HD, SB], BF16, tag="alo",
                                                    name=f"alo_{iB}_{hp}")
                                nc.vector.tensor_mul(alo, t65[h][0:HD, :], bc_t)
                                nc.sync.dma_start(out=anp[HD:128, :], in_=alo)

                    stages = [lambda jj=jj: rect(jj) for jj in range(4 * iB)]
                    stages += [diag, rem, norm]
                    return stages

                for hpp in (0, 2):
                    chain_a = make_stages(hpp)
                    chain_b = make_stages(hpp + 1)
                    for st_a, st_b in zip(chain_a, chain_b):
                        st_a()
                        st_b()

                # ---------------- output projection for block iB ----------------
                for ic in range(4):
                    o_t = o_pool.tile([128, D], F32, tag="o")
                    for dh in range(2):
                        po = ps_acc.tile([128, SB], F32, tag="acc", name=f"po_{iB}_{ic}_{dh}")
                        for hp2 in range(4):
                            nc.tensor.matmul(
                                po,
                                anorms[hp2][:, ts(ic, 128)],
                                wot[hp2][:, ts(dh, SB)],
                                start=(hp2 == 0), stop=(hp2 == 3),
                            )
                        nc.vector.tensor_copy(o_t[:, ts(dh, SB)], po)
                        nc.sync.dma_start(
                            out=out_d[iB * SB + ic * 128:iB * SB + (ic + 1) * 128,
                                      ts(dh, SB)],
                            in_=o_t[:, ts(dh, SB)],
                        )

    nc.compile()
    return nc


def kernel(x, W_q, W_k, W_v, W_o):
    global _cached_nc
    if _cached_nc is None:
        _cached_nc = _build()
    nc = _cached_nc

    import ml_dtypes

    x = np.asarray(x, dtype=np.float32)
    W_q = np.asarray(W_q, dtype=np.float32)
    W_k = np.asarray(W_k, dtype=np.float32)
    W_v = np.asarray(W_v, dtype=np.float32)
    W_o = np.asarray(W_o, dtype=np.float32)

    in_maps = []
    for c in range(NCORES):
        b, g = c // 2, c % 2
        cols = slice(g * DC, (g + 1) * DC)
        in_maps.append({
            "xt": np.ascontiguousarray(x[b].T),
            "wq": np.ascontiguousarray(W_q[:, cols]),
            "wk": np.ascontiguousarray(W_k[:, cols]),
            "wv": np.ascontiguousarray(W_v[:, cols]),
            "wot": np.ascontiguousarray(W_o[:, cols].T).astype(ml_dtypes.bfloat16),
        })

    res = run_bass_kernel_spmd(nc, in_maps, list(range(NCORES))).results
    out = np.empty((B, S, D), np.float32)
    for b in range(B):
        out[b] = res[2 * b]["out"] + res[2 * b + 1]["out"]
    return out
